# revision 26
# baseline (speedup 1.0000x reference)
"""FAPE loss kernel for Trainium2 (8 NeuronCores, Bass/Tile) — v2.

Math
----
The reference computes, for frames i and residue-atoms (l, j):

    local[i, lj, d] = sum_c coords[lj, c] * R[i, d, c] - off[i, d]
    d2[i, lj]       = sum_d (pred_local - true_local)^2
    loss            = sum_{i,lj} m[i] * m[l] * min(sqrt(d2), 10) / ((sum m)^2 * 3 + eps) / 10

d2 is a quadratic form in the 7-vector u'[lj] = [pred_coords(3), true_coords(3), 1]:
    d2[i, lj] = sum_{a<=b} mult_ab * u'_a u'_b * Q[i,(a,b)]
so on device it is a K=84 bf16 matmul (three stacked hi/lo cross terms
Qh.Ph + Qh.Pl + Ql.Ph, fp32-grade: residual ~ |Ql||Pl| ~ 1e-3 absolute).

v2 changes vs v1 (31.5us):
1. MASK COMPACTION.  Both the frame axis i and the residue axis l are
   gated by the same mask; v1 computed all 2048x6144 elements and zeroed
   masked columns.  v2 keeps only valid frames/residues: the device
   processes min(nv, 1024) frames (one 128-frame tile per core) x 3*nv
   columns — ~4x less work for nv~1024.  Overflow frames (nv mod 1024,
   when small) are summed exactly on the host in numpy (O(nv) of the
   O(nv^2) total).
2. SCALAR-FIRST POST-PROCESSING.  v1: DVE clamp from PSUM (1x fp32,
   1.19us/KFD) -> ACT sqrt+accum (1.24us/KFD) — two ~1x passes.  v2: ACT
   reads PSUM directly, sqrt -> SBUF bf16 (the one unavoidable 1x pass,
   (172+FD)/1.2ns), then DVE does min(dist,10) + free-axis sum in ONE
   tensor_scalar with accum_out — bf16/SBUF/step-1 keeps all fast perf
   modes (4x_2P: (58+FD/4)/0.96ns).  Steady state is ACT-paced.
3. sqrt(d2 + b), b=4e-3, guards bf16-split cancellation (d2_split can be
   ~-1e-3 where true d2 ~ 0; sqrt(neg) = NaN).  Systematic effect on the
   loss ~ +b/(2*dist) per element ~ 5e-5 relative — far under the 2e-2
   gate (validated vs reference).

Schedule: input [A (tpc*128) | B (3*nv)] per core arrives as ~1-group
DMA pieces on the SP HW-DGE ring so group-g matmuls wait only on piece g;
the ACT table load + bias-const waits are absorbed at t=0 by a dummy
activation chain (bias tile memset by the idle DVE); a standalone
LDWEIGHTS absorbs piece-0's wait on the PE.  PSUM: one (128,1024) fp32
tile per column group, <=8 banks, no reuse for the common nv<=1024 case.

Toolchain constraint: this walrus build allows ONE semaphore wait per
instruction; remaining multi-wait instructions (the Tile exit drain) are
split onto single-wait no-ops by _split_multi_waits.  Tile's entry/exit
all-engine barriers run in sem-only form (no per-engine drains).
"""

import sys

import numpy as np

for _p in ("/opt/trn_rl_repo",):
    if _p not in sys.path:
        sys.path.insert(0, _p)

import ml_dtypes
import concourse.bass as bass
import concourse.tile as tile
from concourse import mybir
from concourse.bass_utils import run_bass_kernel_spmd

L = 2048
N_CORES = 8
K = 28         # 7*8/2 upper-triangle pairs
KS = 3 * K     # 84: three bf16 cross terms stacked on the contraction axis
CHUNK = 512    # matmul N (one PSUM bank)
GROUP = 1024   # cols per ACT/DVE instruction (2 chunks, one 2-bank PSUM tile)
CLAMP = 10.0
B_EPS = 4e-3   # bias under the sqrt; guards split-cancellation negatives

_PAIRS = [(a, b) for a in range(7) for b in range(a, 7)]
_DIAG_COLS = [k for k, (a, b) in enumerate(_PAIRS) if a == b]


def _split(x):
    hi = x.astype(ml_dtypes.bfloat16)
    lo = (x - hi.astype(np.float32)).astype(ml_dtypes.bfloat16)
    return hi, lo


def _host_prep(pred_coords, true_coords, pred_rotation, pred_translation,
               true_rotation, true_translation, idx):
    """Quadratic-form factors for the compacted (valid-only) problem.

    Returns (B (84, 3*nv) bf16, A (nv, 84) bf16, Qv (nv, 28) f64,
    P64 (28, 3*nv) f64).  All O(L) flops."""
    pc = np.asarray(pred_coords, np.float64)[idx]
    tc = np.asarray(true_coords, np.float64)[idx]
    pR = np.asarray(pred_rotation, np.float64)[idx]
    pT = np.asarray(pred_translation, np.float64)[idx]
    tR = np.asarray(true_rotation, np.float64)[idx]
    tT = np.asarray(true_translation, np.float64)[idx]
    nv = len(idx)

    UT = np.concatenate([
        pc.reshape(nv * 3, 3).T,
        tc.reshape(nv * 3, 3).T,
        np.ones((1, nv * 3)),
    ], axis=0)  # (7, 3*nv)

    offp = np.einsum('ic,idc->id', pT, pR)
    offt = np.einsum('ic,idc->id', tT, tR)
    W = np.concatenate([pR, -tR, -(offp - offt)[:, :, None]], axis=2)  # (nv, 3, 7)
    Q = np.einsum('ida,idb->iab', W, W)  # (nv, 7, 7)

    Qv = np.stack([Q[:, a, b] * (1.0 if a == b else 2.0) for (a, b) in _PAIRS],
                  axis=1)  # (nv, 28) f64
    P64 = np.stack([UT[a] * UT[b] for (a, b) in _PAIRS], axis=0)  # (28, 3*nv)

    Ph, Pl = _split(P64.astype(np.float32))
    Qh, Ql = _split(Qv.astype(np.float32))
    B = np.concatenate([Ph, Pl, Ph], axis=0)   # (84, 3*nv)
    A = np.concatenate([Qh, Qh, Ql], axis=1)   # (nv, 84)
    return np.ascontiguousarray(B), np.ascontiguousarray(A), Qv, P64


def _split_multi_waits(nc):
    """The TPB instruction encodings used by this walrus build carry a single
    semaphore wait.  Tile can emit several waits on one instruction (notably
    the kernel-tail drain).  Split the extras onto same-engine no-ops placed
    immediately before the instruction — engine-order execution makes this
    semantically identical."""
    for bbw in nc.main_func.blocks:
        il = bbw.instructions
        out = []
        changed = False
        for ins in il:
            si = ins.sync_info
            if si is not None and len(si.on_wait) > 1:
                waits = list(si.on_wait)
                for idx_, w in enumerate(waits[:-1]):
                    out.append(mybir.InstNoOp(
                        name=f"{ins.name}-waitsplit{idx_}",
                        engine=ins.engine,
                        sync_info=mybir.SyncInfo(on_wait=[w], on_update=[]),
                    ))
                si.on_wait = [waits[-1]]
                changed = True
            out.append(ins)
        if changed:
            bbw.instructions = out


def _hoist_preamble(nc):
    """Move the wait-free head of the kernel body — input-DMA issues, the
    bias memset, and the dummy-activation chain (which drags the ~1.3us ACT
    table load with it) — into the preamble block, ahead of the Tile entry
    barrier.  The entry barrier completes ~1.5us after the per-engine
    register-init moves; these instructions depend on none of the state the
    barrier protects (fresh SBUF tiles, runtime-cleared semaphores), so
    hoisting starts the input transfer and the table load that much sooner.
    Per-engine program order is preserved; cross-engine order is already
    carried by the semaphores the instructions themselves hold."""
    blocks = nc.main_func.blocks
    if len(blocks) < 2:
        return
    pre, body = blocks[0], blocks[1]

    hoist = []
    n_act = 0
    for ins in body.instructions:
        eng = ins.engine
        if isinstance(ins, mybir.InstDMACopy) and not (
                ins.sync_info and ins.sync_info.on_wait):
            hoist.append(ins)
        elif isinstance(ins, mybir.InstMemset) and eng == mybir.EngineType.DVE:
            hoist.append(ins)
        elif isinstance(ins, mybir.InstActivation) and n_act < 2:
            # the two dummy activations are the first ACT instructions
            hoist.append(ins)
            n_act += 1
    body.instructions = [i for i in body.instructions if i not in hoist]

    # Insertion point per engine: before that engine's first entry-barrier
    # event (fallback: end of preamble).
    out = []
    inserted = set()
    for ins in pre.instructions:
        if (isinstance(ins, mybir.InstEventSemaphore)
                and ins.engine not in inserted):
            out.extend(h for h in hoist if h.engine == ins.engine)
            inserted.add(ins.engine)
        out.append(ins)
    for h in hoist:
        if h.engine not in inserted:
            out.append(h)
    pre.instructions = out


def _build_program(tpc, widths, split_waits=True):
    """tpc: frame tiles per core (usually 1); widths: per-group column
    widths (each <= GROUP, even)."""
    _orig_aeb = bass.Bass.all_engine_barrier
    bass.Bass.all_engine_barrier = (
        lambda self, *, sem_only=False: _orig_aeb(self, sem_only=True))
    try:
        nc = _build_program_inner(tpc, widths)
    finally:
        bass.Bass.all_engine_barrier = _orig_aeb
    _hoist_preamble(nc)
    if split_waits:
        _split_multi_waits(nc)
    return nc


def _build_program_inner(tpc, widths):
    f32 = mybir.dt.float32
    bf16 = mybir.dt.bfloat16
    C = sum(widths)
    A_COLS = tpc * 128
    NGROUPS = tpc * len(widths)
    nW = len(widths)

    nc = bass.Bass()
    inp = nc.declare_dram_parameter("inp", [KS, A_COLS + C], bf16,
                                    isOutput=False)
    fsums = nc.declare_dram_parameter("fsums", [128, NGROUPS], f32,
                                      isOutput=True)

    # Input DMA pieces: piece 0 (SP ring) = all A tiles + the first chunk
    # (small, so its completion receipt — the ~2.4us critical-path gate for
    # the first matmul — starts as early as possible); then two-chunk pieces
    # split between the SP and GPSIMD (SWDGE) rings so transfers overlap.
    bounds = [0, A_COLS + widths[0]]
    off = bounds[-1]
    for gi in range(1, nW):
        off += widths[gi]
        if gi + 1 < nW and widths[gi + 1] < CHUNK:
            continue  # tiny trailing group rides with its predecessor
        bounds.append(off)
    bounds[-1] = A_COLS + C
    n_pieces = len(bounds) - 1
    # Every piece pays its own ~2.5-3.5us issue->completion-sem latency, so
    # pieces issue in parallel across the SP HWDGE and GPSIMD SWDGE rings at
    # t~0 — piece 0 (the matmul gate) alone and first on SP, the next pieces
    # on GPSIMD, the rest back on SP.  The ACT ring stays clear: a piece
    # there would delay the sqrt table load (and its SDMA traffic delays
    # piece 0).
    n_gp = (n_pieces - 1 + 1) // 2
    piece_engines = (["sync"] + ["gpsimd"] * n_gp
                     + ["sync"] * max(0, n_pieces - 1 - n_gp))

    # acc is split into two tiles: the first output DMA (all but the last
    # two groups) issues mid-pipeline and its ~2us HBM-write receipt hides
    # under compute; only the final ~1KB DMA's receipt sits on the tail.
    # (Tile tracks deps per-tile, so the split avoids false dependencies.)
    n_acc_a = max(1, NGROUPS - 2)
    v_cols = list(range(NGROUPS))

    with tile.TileContext(nc) as tc:
        with tc.tile_pool(name="const", bufs=1) as const_pool, \
             tc.tile_pool(name="dist", bufs=2 * NGROUPS) as dist_pool, \
             tc.tile_pool(name="ps",
                          bufs=min(NGROUPS, 8 * CHUNK // max(widths)),
                          space="PSUM") as ps:
            data = const_pool.tile([KS, A_COLS + C], bf16)
            for i in range(len(bounds) - 1):
                eng = getattr(nc, piece_engines[i])
                eng.dma_start(data[:, bounds[i]:bounds[i + 1]],
                              inp[:, bounds[i]:bounds[i + 1]])

            acc_a = const_pool.tile([128, n_acc_a], f32)
            if NGROUPS > n_acc_a:
                acc_b = const_pool.tile([128, NGROUPS - n_acc_a], f32)
            else:
                acc_b = None
            bias_t = const_pool.tile([128, 1], f32)
            scratch_t = const_pool.tile([128, 1], f32)
            # Idle DVE fills the sqrt-bias const; the dummy activation chain
            # on ACT absorbs the bias wait AND triggers the sqrt table load
            # (PWP, ~1.3us) at t~0, hidden under the input DMA.
            nc.vector.memset(bias_t[:], B_EPS)
            nc.scalar.activation(scratch_t[:], bias_t[:],
                                 mybir.ActivationFunctionType.Sqrt,
                                 bias=bias_t[:, 0:1])
            nc.scalar.activation(scratch_t[:], bias_t[:],
                                 mybir.ActivationFunctionType.Sqrt,
                                 bias=bias_t[:, 0:1])

            for t in range(tpc):
                col = A_COLS
                for gi, w in enumerate(widths):
                    g = t * nW + gi
                    if gi == 0:
                        # Standalone LDWEIGHTS absorbs the DMA-queue wait on
                        # the PE so the real matmuls only wait on their PSUM
                        # slot (single-wait-per-instruction limit).
                        nc.tensor.ldweights(data[:, t * 128:t * 128 + 128])
                    d2 = ps.tile([128, w], f32, tag="d2")
                    for c0 in range(0, w, CHUNK):
                        cw = min(CHUNK, w - c0)
                        nc.tensor.matmul(
                            d2[:, c0:c0 + cw],
                            data[:, t * 128:(t + 1) * 128],
                            data[:, col + c0:col + c0 + cw],
                            start=True, stop=True,
                        )
                    dist = dist_pool.tile([128, w], bf16, tag="dist")
                    nc.scalar.activation(
                        dist[:], d2[:],
                        mybir.ActivationFunctionType.Sqrt,
                        bias=bias_t[:, 0:1],
                    )
                    clamp = dist_pool.tile([128, w], bf16, tag="clamp")
                    # With accum_out, op1 is the reduction op (walrus
                    # TensorScalarPtrReduce form): out = min(in0, 10),
                    # accum_out = sum(out).
                    if g < n_acc_a:
                        acc_ap = acc_a[:, g:g + 1]
                    else:
                        acc_ap = acc_b[:, g - n_acc_a:g - n_acc_a + 1]
                    nc.vector.tensor_scalar(
                        out=clamp[:], in0=dist[:],
                        scalar1=CLAMP, scalar2=None,
                        op0=mybir.AluOpType.min,
                        op1=mybir.AluOpType.add,
                        accum_out=acc_ap,
                    )
                    col += w

            nc.sync.dma_start(fsums[:, 0:n_acc_a], acc_a[:])
            if acc_b is not None:
                nc.sync.dma_start(fsums[:, n_acc_a:], acc_b[:])
    return nc


_PROGRAM_CACHE = {}


def _get_program(tpc, widths):
    key = (tpc, tuple(widths))
    if key not in _PROGRAM_CACHE:
        _PROGRAM_CACHE[key] = _build_program(tpc, widths)
    return _PROGRAM_CACHE[key]


def kernel(pred_coords, true_coords, pred_rotation, pred_translation,
           true_rotation, true_translation, mask, **_run_kwargs):
    mask = np.asarray(mask)
    idx = np.nonzero(mask != 0)[0]
    nv = len(idx)
    if nv == 0:
        out = np.float32(0.0)
        return (out, None) if _run_kwargs else out

    B, A, Qv, P64 = _host_prep(pred_coords, true_coords, pred_rotation,
                               pred_translation, true_rotation,
                               true_translation, idx)
    C = 3 * nv

    # Frame split: device takes n_dev = tpc*1024 (one or more full 128-frame
    # tiles per core); a small overflow is summed exactly on the host.
    tpc = max(1, int(round(nv / 1024)))
    n_dev = min(nv, tpc * 1024)
    n_off = nv - n_dev

    # Column groups: two CHUNK-wide groups first (small, so the ACT/DVE
    # pipeline starts as soon as the first chunk's matmul lands), then
    # 2*CHUNK-wide groups (halved per-instruction overhead), remainder last
    # (kept even for the DVE 2-byte perf modes).
    C_pad = C + (C & 1)
    widths = []
    rest = C_pad
    while rest > 0:
        if len(widths) < 2:
            w = min(CHUNK, rest)
        else:
            w = min(2 * CHUNK, rest)
        if rest - w < 0:
            w = rest
        widths.append(w)
        rest -= w
    NGROUPS = tpc * len(widths)

    A_pad = np.zeros((tpc * 1024, KS), A.dtype)
    A_pad[:n_dev] = A[:n_dev]
    B_pad = np.zeros((KS, C_pad), B.dtype)
    B_pad[:, :C] = B

    in_maps = []
    for c in range(N_CORES):
        a_c = A_pad[c * tpc * 128:(c + 1) * tpc * 128].T  # (84, tpc*128)
        in_maps.append({"inp": np.ascontiguousarray(
            np.concatenate([a_c, B_pad], axis=1))})

    nc = _get_program(tpc, widths)
    res = run_bass_kernel_spmd(nc, in_maps, list(range(N_CORES)),
                               **_run_kwargs)

    # fsums column j = group j; every group of frame-tile t contributes to
    # the same partitions, so the sum only needs each column's tile index.
    nW = len(widths)
    col_tile = [g // nW for g in range(NGROUPS)]

    numer = 0.0
    sqrt_b = float(np.sqrt(B_EPS))  # pad-column contribution (approx; tiny)
    n_pad_cols = C_pad - C
    for c in range(N_CORES):
        fs = np.asarray(res.results[c]["fsums"], np.float64)  # (128, NGROUPS)
        for j, t in enumerate(col_tile):
            f0 = (c * tpc + t) * 128
            cnt = min(128, max(0, n_dev - f0))
            if cnt == 0:
                continue
            numer += fs[:cnt, j].sum()
        # pad columns (at most one, when 3*nv is odd) sit in the last group
        # of every tile; subtract their sqrt(B_EPS) contribution.
        if n_pad_cols:
            for t in range(tpc):
                f0 = (c * tpc + t) * 128
                cnt = min(128, max(0, n_dev - f0))
                numer -= cnt * n_pad_cols * sqrt_b

    if n_off:
        # Exact host sum for the overflow frames (O(n_off * 3nv) elements).
        d2 = Qv[n_dev:nv] @ P64  # (n_off, 3*nv)
        numer += np.minimum(np.sqrt(np.maximum(d2, 0.0)), CLAMP).sum()

    denom = float(nv) ** 2 * 3.0 + 1e-8
    out = np.float32(numer / denom / 10.0)
    if _run_kwargs:
        return out, res
    return out


# revision 27
# speedup vs baseline: 1.0132x; 1.0132x over previous
"""FAPE loss kernel for Trainium2 (8 NeuronCores, Bass/Tile) — v2.

Math
----
The reference computes, for frames i and residue-atoms (l, j):

    local[i, lj, d] = sum_c coords[lj, c] * R[i, d, c] - off[i, d]
    d2[i, lj]       = sum_d (pred_local - true_local)^2
    loss            = sum_{i,lj} m[i] * m[l] * min(sqrt(d2), 10) / ((sum m)^2 * 3 + eps) / 10

d2 is a quadratic form in the 7-vector u'[lj] = [pred_coords(3), true_coords(3), 1]:
    d2[i, lj] = sum_{a<=b} mult_ab * u'_a u'_b * Q[i,(a,b)]
so on device it is a K=84 bf16 matmul (three stacked hi/lo cross terms
Qh.Ph + Qh.Pl + Ql.Ph, fp32-grade: residual ~ |Ql||Pl| ~ 1e-3 absolute).

v2 changes vs v1 (31.5us):
1. MASK COMPACTION.  Both the frame axis i and the residue axis l are
   gated by the same mask; v1 computed all 2048x6144 elements and zeroed
   masked columns.  v2 keeps only valid frames/residues: the device
   processes min(nv, 1024) frames (one 128-frame tile per core) x 3*nv
   columns — ~4x less work for nv~1024.  Overflow frames (nv mod 1024,
   when small) are summed exactly on the host in numpy (O(nv) of the
   O(nv^2) total).
2. SCALAR-FIRST POST-PROCESSING.  v1: DVE clamp from PSUM (1x fp32,
   1.19us/KFD) -> ACT sqrt+accum (1.24us/KFD) — two ~1x passes.  v2: ACT
   reads PSUM directly, sqrt -> SBUF bf16 (the one unavoidable 1x pass,
   (172+FD)/1.2ns), then DVE does min(dist,10) + free-axis sum in ONE
   tensor_scalar with accum_out — bf16/SBUF/step-1 keeps all fast perf
   modes (4x_2P: (58+FD/4)/0.96ns).  Steady state is ACT-paced.
3. sqrt(d2 + b), b=4e-3, guards bf16-split cancellation (d2_split can be
   ~-1e-3 where true d2 ~ 0; sqrt(neg) = NaN).  Systematic effect on the
   loss ~ +b/(2*dist) per element ~ 5e-5 relative — far under the 2e-2
   gate (validated vs reference).

Schedule: input [A (tpc*128) | B (3*nv)] per core arrives as ~1-group
DMA pieces on the SP HW-DGE ring so group-g matmuls wait only on piece g;
the ACT table load + bias-const waits are absorbed at t=0 by a dummy
activation chain (bias tile memset by the idle DVE); a standalone
LDWEIGHTS absorbs piece-0's wait on the PE.  PSUM: one (128,1024) fp32
tile per column group, <=8 banks, no reuse for the common nv<=1024 case.

Toolchain constraint: this walrus build allows ONE semaphore wait per
instruction; remaining multi-wait instructions (the Tile exit drain) are
split onto single-wait no-ops by _split_multi_waits.  Tile's entry/exit
all-engine barriers run in sem-only form (no per-engine drains).
"""

import sys

import numpy as np

for _p in ("/opt/trn_rl_repo",):
    if _p not in sys.path:
        sys.path.insert(0, _p)

import ml_dtypes
import concourse.bass as bass
import concourse.tile as tile
from concourse import mybir
from concourse.bass_utils import run_bass_kernel_spmd

L = 2048
N_CORES = 8
K = 28         # 7*8/2 upper-triangle pairs
KS = 3 * K     # 84: three bf16 cross terms stacked on the contraction axis
CHUNK = 512    # matmul N (one PSUM bank)
GROUP = 1024   # cols per ACT/DVE instruction (2 chunks, one 2-bank PSUM tile)
CLAMP = 10.0
B_EPS = 4e-3   # bias under the sqrt; guards split-cancellation negatives

_PAIRS = [(a, b) for a in range(7) for b in range(a, 7)]
_DIAG_COLS = [k for k, (a, b) in enumerate(_PAIRS) if a == b]


def _split(x):
    hi = x.astype(ml_dtypes.bfloat16)
    lo = (x - hi.astype(np.float32)).astype(ml_dtypes.bfloat16)
    return hi, lo


def _host_prep(pred_coords, true_coords, pred_rotation, pred_translation,
               true_rotation, true_translation, idx):
    """Quadratic-form factors for the compacted (valid-only) problem.

    Returns (B (84, 3*nv) bf16, A (nv, 84) bf16, Qv (nv, 28) f64,
    P64 (28, 3*nv) f64).  All O(L) flops."""
    pc = np.asarray(pred_coords, np.float64)[idx]
    tc = np.asarray(true_coords, np.float64)[idx]
    pR = np.asarray(pred_rotation, np.float64)[idx]
    pT = np.asarray(pred_translation, np.float64)[idx]
    tR = np.asarray(true_rotation, np.float64)[idx]
    tT = np.asarray(true_translation, np.float64)[idx]
    nv = len(idx)

    UT = np.concatenate([
        pc.reshape(nv * 3, 3).T,
        tc.reshape(nv * 3, 3).T,
        np.ones((1, nv * 3)),
    ], axis=0)  # (7, 3*nv)

    offp = np.einsum('ic,idc->id', pT, pR)
    offt = np.einsum('ic,idc->id', tT, tR)
    W = np.concatenate([pR, -tR, -(offp - offt)[:, :, None]], axis=2)  # (nv, 3, 7)
    Q = np.einsum('ida,idb->iab', W, W)  # (nv, 7, 7)

    Qv = np.stack([Q[:, a, b] * (1.0 if a == b else 2.0) for (a, b) in _PAIRS],
                  axis=1)  # (nv, 28) f64
    P64 = np.stack([UT[a] * UT[b] for (a, b) in _PAIRS], axis=0)  # (28, 3*nv)

    Ph, Pl = _split(P64.astype(np.float32))
    Qh, Ql = _split(Qv.astype(np.float32))
    B = np.concatenate([Ph, Pl, Ph], axis=0)   # (84, 3*nv)
    A = np.concatenate([Qh, Qh, Ql], axis=1)   # (nv, 84)
    return np.ascontiguousarray(B), np.ascontiguousarray(A), Qv, P64


def _split_multi_waits(nc):
    """The TPB instruction encodings used by this walrus build carry a single
    semaphore wait.  Tile can emit several waits on one instruction (notably
    the kernel-tail drain).  Split the extras onto same-engine no-ops placed
    immediately before the instruction — engine-order execution makes this
    semantically identical."""
    for bbw in nc.main_func.blocks:
        il = bbw.instructions
        out = []
        changed = False
        for ins in il:
            si = ins.sync_info
            if si is not None and len(si.on_wait) > 1:
                waits = list(si.on_wait)
                for idx_, w in enumerate(waits[:-1]):
                    out.append(mybir.InstNoOp(
                        name=f"{ins.name}-waitsplit{idx_}",
                        engine=ins.engine,
                        sync_info=mybir.SyncInfo(on_wait=[w], on_update=[]),
                    ))
                si.on_wait = [waits[-1]]
                changed = True
            out.append(ins)
        if changed:
            bbw.instructions = out


def _hoist_preamble(nc):
    """Move the wait-free head of the kernel body — input-DMA issues, the
    bias memset, and the dummy-activation chain (which drags the ~1.3us ACT
    table load with it) — into the preamble block, ahead of the Tile entry
    barrier.  The entry barrier completes ~1.5us after the per-engine
    register-init moves; these instructions depend on none of the state the
    barrier protects (fresh SBUF tiles, runtime-cleared semaphores), so
    hoisting starts the input transfer and the table load that much sooner.
    Per-engine program order is preserved; cross-engine order is already
    carried by the semaphores the instructions themselves hold."""
    blocks = nc.main_func.blocks
    if len(blocks) < 2:
        return
    pre, body = blocks[0], blocks[1]

    hoist = []
    n_act = 0
    for ins in body.instructions:
        eng = ins.engine
        if isinstance(ins, mybir.InstDMACopy) and not (
                ins.sync_info and ins.sync_info.on_wait):
            hoist.append(ins)
        elif isinstance(ins, mybir.InstMemset) and eng == mybir.EngineType.DVE:
            hoist.append(ins)
        elif isinstance(ins, mybir.InstActivation) and n_act < 2:
            # the two dummy activations are the first ACT instructions
            hoist.append(ins)
            n_act += 1
    body.instructions = [i for i in body.instructions if i not in hoist]

    # Insertion point per engine: before that engine's first entry-barrier
    # event (fallback: end of preamble).
    out = []
    inserted = set()
    for ins in pre.instructions:
        if (isinstance(ins, mybir.InstEventSemaphore)
                and ins.engine not in inserted):
            out.extend(h for h in hoist if h.engine == ins.engine)
            inserted.add(ins.engine)
        out.append(ins)
    for h in hoist:
        if h.engine not in inserted:
            out.append(h)
    pre.instructions = out


def _build_program(tpc, widths, split_waits=True):
    """tpc: frame tiles per core (usually 1); widths: per-group column
    widths (each <= GROUP, even)."""
    _orig_aeb = bass.Bass.all_engine_barrier
    bass.Bass.all_engine_barrier = (
        lambda self, *, sem_only=False: _orig_aeb(self, sem_only=True))
    try:
        nc = _build_program_inner(tpc, widths)
    finally:
        bass.Bass.all_engine_barrier = _orig_aeb
    _hoist_preamble(nc)
    if split_waits:
        _split_multi_waits(nc)
    return nc


def _build_program_inner(tpc, widths):
    f32 = mybir.dt.float32
    bf16 = mybir.dt.bfloat16
    C = sum(widths)
    A_COLS = tpc * 128
    NGROUPS = tpc * len(widths)
    nW = len(widths)

    nc = bass.Bass()
    inp = nc.declare_dram_parameter("inp", [KS, A_COLS + C], bf16,
                                    isOutput=False)
    fsums = nc.declare_dram_parameter("fsums", [128, NGROUPS], f32,
                                      isOutput=True)

    # Input DMA pieces: piece 0 (SP ring) = all A tiles + the first chunk
    # (small, so its completion receipt — the ~2.4us critical-path gate for
    # the first matmul — starts as early as possible); then two-chunk pieces
    # split between the SP and GPSIMD (SWDGE) rings so transfers overlap.
    bounds = [0, A_COLS + sum(widths[0:2])]
    off = bounds[-1]
    for gi in range(2, nW):
        off += widths[gi]
        if gi + 1 < nW and widths[gi + 1] < CHUNK:
            continue  # tiny trailing group rides with its predecessor
        bounds.append(off)
    bounds[-1] = A_COLS + C
    n_pieces = len(bounds) - 1
    # Every piece pays its own ~2.5-3.5us issue->completion-sem latency, so
    # pieces issue in parallel across the SP HWDGE and GPSIMD SWDGE rings at
    # t~0: piece 0 (A + the first two groups — the pipeline head) first on
    # SP, later groups one piece each, alternating GPSIMD/SP so each ring's
    # first piece lands early.  The ACT ring stays clear: a piece there
    # would delay the sqrt table load (and its SDMA traffic delays piece 0).
    piece_engines = ["sync"] + ["gpsimd", "sync"] * n_pieces
    piece_engines = piece_engines[:n_pieces]

    # acc is split into two tiles: the first output DMA (all but the last
    # two groups) issues mid-pipeline and its ~2us HBM-write receipt hides
    # under compute; only the final ~1KB DMA's receipt sits on the tail.
    # (Tile tracks deps per-tile, so the split avoids false dependencies.)
    n_acc_a = max(1, NGROUPS - 2)
    v_cols = list(range(NGROUPS))

    with tile.TileContext(nc) as tc:
        with tc.tile_pool(name="const", bufs=1) as const_pool, \
             tc.tile_pool(name="dist", bufs=2 * NGROUPS) as dist_pool, \
             tc.tile_pool(name="ps",
                          bufs=min(NGROUPS, 8 * CHUNK // max(widths)),
                          space="PSUM") as ps:
            data = const_pool.tile([KS, A_COLS + C], bf16)
            for i in range(len(bounds) - 1):
                eng = getattr(nc, piece_engines[i])
                eng.dma_start(data[:, bounds[i]:bounds[i + 1]],
                              inp[:, bounds[i]:bounds[i + 1]])

            acc_a = const_pool.tile([128, n_acc_a], f32)
            if NGROUPS > n_acc_a:
                acc_b = const_pool.tile([128, NGROUPS - n_acc_a], f32)
            else:
                acc_b = None
            bias_t = const_pool.tile([128, 1], f32)
            scratch_t = const_pool.tile([128, 1], f32)
            # Idle DVE fills the sqrt-bias const; the dummy activation chain
            # on ACT absorbs the bias wait AND triggers the sqrt table load
            # (PWP, ~1.3us) at t~0, hidden under the input DMA.
            nc.vector.memset(bias_t[:], B_EPS)
            nc.scalar.activation(scratch_t[:], bias_t[:],
                                 mybir.ActivationFunctionType.Sqrt,
                                 bias=bias_t[:, 0:1])
            nc.scalar.activation(scratch_t[:], bias_t[:],
                                 mybir.ActivationFunctionType.Sqrt,
                                 bias=bias_t[:, 0:1])

            for t in range(tpc):
                col = A_COLS
                for gi, w in enumerate(widths):
                    g = t * nW + gi
                    if gi == 0:
                        # Standalone LDWEIGHTS absorbs the DMA-queue wait on
                        # the PE so the real matmuls only wait on their PSUM
                        # slot (single-wait-per-instruction limit).
                        nc.tensor.ldweights(data[:, t * 128:t * 128 + 128])
                    d2 = ps.tile([128, w], f32, tag="d2")
                    for c0 in range(0, w, CHUNK):
                        cw = min(CHUNK, w - c0)
                        nc.tensor.matmul(
                            d2[:, c0:c0 + cw],
                            data[:, t * 128:(t + 1) * 128],
                            data[:, col + c0:col + c0 + cw],
                            start=True, stop=True,
                        )
                    dist = dist_pool.tile([128, w], bf16, tag="dist")
                    nc.scalar.activation(
                        dist[:], d2[:],
                        mybir.ActivationFunctionType.Sqrt,
                        bias=bias_t[:, 0:1],
                    )
                    clamp = dist_pool.tile([128, w], bf16, tag="clamp")
                    # With accum_out, op1 is the reduction op (walrus
                    # TensorScalarPtrReduce form): out = min(in0, 10),
                    # accum_out = sum(out).
                    if g < n_acc_a:
                        acc_ap = acc_a[:, g:g + 1]
                    else:
                        acc_ap = acc_b[:, g - n_acc_a:g - n_acc_a + 1]
                    nc.vector.tensor_scalar(
                        out=clamp[:], in0=dist[:],
                        scalar1=CLAMP, scalar2=None,
                        op0=mybir.AluOpType.min,
                        op1=mybir.AluOpType.add,
                        accum_out=acc_ap,
                    )
                    col += w

            nc.sync.dma_start(fsums[:, 0:n_acc_a], acc_a[:])
            if acc_b is not None:
                nc.sync.dma_start(fsums[:, n_acc_a:], acc_b[:])
    return nc


_PROGRAM_CACHE = {}


def _get_program(tpc, widths):
    key = (tpc, tuple(widths))
    if key not in _PROGRAM_CACHE:
        _PROGRAM_CACHE[key] = _build_program(tpc, widths)
    return _PROGRAM_CACHE[key]


def kernel(pred_coords, true_coords, pred_rotation, pred_translation,
           true_rotation, true_translation, mask, **_run_kwargs):
    mask = np.asarray(mask)
    idx = np.nonzero(mask != 0)[0]
    nv = len(idx)
    if nv == 0:
        out = np.float32(0.0)
        return (out, None) if _run_kwargs else out

    B, A, Qv, P64 = _host_prep(pred_coords, true_coords, pred_rotation,
                               pred_translation, true_rotation,
                               true_translation, idx)
    C = 3 * nv

    # Frame split: device takes n_dev = tpc*1024 (one or more full 128-frame
    # tiles per core); a small overflow is summed exactly on the host.
    tpc = max(1, int(round(nv / 1024)))
    n_dev = min(nv, tpc * 1024)
    n_off = nv - n_dev

    # Column groups: two CHUNK-wide groups first (small, so the ACT/DVE
    # pipeline starts as soon as the first chunk's matmul lands), then
    # 2*CHUNK-wide groups (halved per-instruction overhead), remainder last
    # (kept even for the DVE 2-byte perf modes).
    C_pad = C + (C & 1)
    widths = []
    rest = C_pad
    while rest > 0:
        if len(widths) < 2:
            w = min(CHUNK, rest)
        else:
            w = min(2 * CHUNK, rest)
        if rest - w < 0:
            w = rest
        widths.append(w)
        rest -= w
    NGROUPS = tpc * len(widths)

    A_pad = np.zeros((tpc * 1024, KS), A.dtype)
    A_pad[:n_dev] = A[:n_dev]
    B_pad = np.zeros((KS, C_pad), B.dtype)
    B_pad[:, :C] = B

    in_maps = []
    for c in range(N_CORES):
        a_c = A_pad[c * tpc * 128:(c + 1) * tpc * 128].T  # (84, tpc*128)
        in_maps.append({"inp": np.ascontiguousarray(
            np.concatenate([a_c, B_pad], axis=1))})

    nc = _get_program(tpc, widths)
    res = run_bass_kernel_spmd(nc, in_maps, list(range(N_CORES)),
                               **_run_kwargs)

    # fsums column j = group j; every group of frame-tile t contributes to
    # the same partitions, so the sum only needs each column's tile index.
    nW = len(widths)
    col_tile = [g // nW for g in range(NGROUPS)]

    numer = 0.0
    sqrt_b = float(np.sqrt(B_EPS))  # pad-column contribution (approx; tiny)
    n_pad_cols = C_pad - C
    for c in range(N_CORES):
        fs = np.asarray(res.results[c]["fsums"], np.float64)  # (128, NGROUPS)
        for j, t in enumerate(col_tile):
            f0 = (c * tpc + t) * 128
            cnt = min(128, max(0, n_dev - f0))
            if cnt == 0:
                continue
            numer += fs[:cnt, j].sum()
        # pad columns (at most one, when 3*nv is odd) sit in the last group
        # of every tile; subtract their sqrt(B_EPS) contribution.
        if n_pad_cols:
            for t in range(tpc):
                f0 = (c * tpc + t) * 128
                cnt = min(128, max(0, n_dev - f0))
                numer -= cnt * n_pad_cols * sqrt_b

    if n_off:
        # Exact host sum for the overflow frames (O(n_off * 3nv) elements).
        d2 = Qv[n_dev:nv] @ P64  # (n_off, 3*nv)
        numer += np.minimum(np.sqrt(np.maximum(d2, 0.0)), CLAMP).sum()

    denom = float(nv) ** 2 * 3.0 + 1e-8
    out = np.float32(numer / denom / 10.0)
    if _run_kwargs:
        return out, res
    return out


# revision 32
# speedup vs baseline: 1.0391x; 1.0257x over previous
"""FAPE loss kernel for Trainium2 (8 NeuronCores, Bass/Tile) — v2.

Math
----
The reference computes, for frames i and residue-atoms (l, j):

    local[i, lj, d] = sum_c coords[lj, c] * R[i, d, c] - off[i, d]
    d2[i, lj]       = sum_d (pred_local - true_local)^2
    loss            = sum_{i,lj} m[i] * m[l] * min(sqrt(d2), 10) / ((sum m)^2 * 3 + eps) / 10

d2 is a quadratic form in the 7-vector u'[lj] = [pred_coords(3), true_coords(3), 1]:
    d2[i, lj] = sum_{a<=b} mult_ab * u'_a u'_b * Q[i,(a,b)]
so on device it is a K=84 bf16 matmul (three stacked hi/lo cross terms
Qh.Ph + Qh.Pl + Ql.Ph, fp32-grade: residual ~ |Ql||Pl| ~ 1e-3 absolute).

v2 changes vs v1 (31.5us):
1. MASK COMPACTION.  Both the frame axis i and the residue axis l are
   gated by the same mask; v1 computed all 2048x6144 elements and zeroed
   masked columns.  v2 keeps only valid frames/residues: the device
   processes min(nv, 1024) frames (one 128-frame tile per core) x 3*nv
   columns — ~4x less work for nv~1024.  Overflow frames (nv mod 1024,
   when small) are summed exactly on the host in numpy (O(nv) of the
   O(nv^2) total).
2. SCALAR-FIRST POST-PROCESSING.  v1: DVE clamp from PSUM (1x fp32,
   1.19us/KFD) -> ACT sqrt+accum (1.24us/KFD) — two ~1x passes.  v2: ACT
   reads PSUM directly, sqrt -> SBUF bf16 (the one unavoidable 1x pass,
   (172+FD)/1.2ns), then DVE does min(dist,10) + free-axis sum in ONE
   tensor_scalar with accum_out — bf16/SBUF/step-1 keeps all fast perf
   modes (4x_2P: (58+FD/4)/0.96ns).  Steady state is ACT-paced.
3. sqrt(d2 + b), b=4e-3, guards bf16-split cancellation (d2_split can be
   ~-1e-3 where true d2 ~ 0; sqrt(neg) = NaN).  Systematic effect on the
   loss ~ +b/(2*dist) per element ~ 5e-5 relative — far under the 2e-2
   gate (validated vs reference).

Schedule: input [A (tpc*128) | B (3*nv)] per core arrives as ~1-group
DMA pieces on the SP HW-DGE ring so group-g matmuls wait only on piece g;
the ACT table load + bias-const waits are absorbed at t=0 by a dummy
activation chain (bias tile memset by the idle DVE); a standalone
LDWEIGHTS absorbs piece-0's wait on the PE.  PSUM: one (128,1024) fp32
tile per column group, <=8 banks, no reuse for the common nv<=1024 case.

Toolchain constraint: this walrus build allows ONE semaphore wait per
instruction; remaining multi-wait instructions (the Tile exit drain) are
split onto single-wait no-ops by _split_multi_waits.  Tile's entry/exit
all-engine barriers run in sem-only form (no per-engine drains).
"""

import sys

import numpy as np

for _p in ("/opt/trn_rl_repo",):
    if _p not in sys.path:
        sys.path.insert(0, _p)

import ml_dtypes
import concourse.bass as bass
import concourse.tile as tile
from concourse import mybir
from concourse.bass_utils import run_bass_kernel_spmd

L = 2048
N_CORES = 8
K = 28         # 7*8/2 upper-triangle pairs
KS = 2 * K     # 56: two bf16 cross terms (Qh.Ph + Qh.Pl) on the contraction
CHUNK = 512    # matmul N (one PSUM bank)
GROUP = 1024   # cols per ACT/DVE instruction (2 chunks, one 2-bank PSUM tile)
CLAMP = 10.0
CLAMP2 = 100.0

_PAIRS = [(a, b) for a in range(7) for b in range(a, 7)]
_DIAG_COLS = [k for k, (a, b) in enumerate(_PAIRS) if a == b]


def _split(x):
    hi = x.astype(ml_dtypes.bfloat16)
    lo = (x - hi.astype(np.float32)).astype(ml_dtypes.bfloat16)
    return hi, lo


def _host_prep(pred_coords, true_coords, pred_rotation, pred_translation,
               true_rotation, true_translation, idx):
    """Quadratic-form factors for the compacted (valid-only) problem.

    Returns (B (84, 3*nv) bf16, A (nv, 84) bf16, Qv (nv, 28) f64,
    P64 (28, 3*nv) f64).  All O(L) flops."""
    pc = np.asarray(pred_coords, np.float64)[idx]
    tc = np.asarray(true_coords, np.float64)[idx]
    pR = np.asarray(pred_rotation, np.float64)[idx]
    pT = np.asarray(pred_translation, np.float64)[idx]
    tR = np.asarray(true_rotation, np.float64)[idx]
    tT = np.asarray(true_translation, np.float64)[idx]
    nv = len(idx)

    UT = np.concatenate([
        pc.reshape(nv * 3, 3).T,
        tc.reshape(nv * 3, 3).T,
        np.ones((1, nv * 3)),
    ], axis=0)  # (7, 3*nv)

    offp = np.einsum('ic,idc->id', pT, pR)
    offt = np.einsum('ic,idc->id', tT, tR)
    W = np.concatenate([pR, -tR, -(offp - offt)[:, :, None]], axis=2)  # (nv, 3, 7)
    Q = np.einsum('ida,idb->iab', W, W)  # (nv, 7, 7)

    Qv = np.stack([Q[:, a, b] * (1.0 if a == b else 2.0) for (a, b) in _PAIRS],
                  axis=1)  # (nv, 28) f64
    P64 = np.stack([UT[a] * UT[b] for (a, b) in _PAIRS], axis=0)  # (28, 3*nv)

    Ph, Pl = _split(P64.astype(np.float32))
    Qh, _ = _split(Qv.astype(np.float32))
    # Two-term split: d2 ~ Qh.(Ph + Pl); the dropped Ql.P term is a ~2e-3
    # relative error with random sign (loss impact ~5e-4, measured), and the
    # device clamps d2 to [0, 100] before the sqrt so the negative residuals
    # near d2=0 are harmless.  Costs 1/3 less DMA than the 3-term split.
    B = np.concatenate([Ph, Pl], axis=0)       # (56, 3*nv)
    A = np.concatenate([Qh, Qh], axis=1)       # (nv, 56)
    return np.ascontiguousarray(B), np.ascontiguousarray(A), Qv, P64


def _split_multi_waits(nc):
    """The TPB instruction encodings used by this walrus build carry a single
    semaphore wait.  Tile can emit several waits on one instruction (notably
    the kernel-tail drain).  Split the extras onto same-engine no-ops placed
    immediately before the instruction — engine-order execution makes this
    semantically identical."""
    for bbw in nc.main_func.blocks:
        il = bbw.instructions
        out = []
        changed = False
        for ins in il:
            si = ins.sync_info
            if si is not None and len(si.on_wait) > 1:
                waits = list(si.on_wait)
                for idx_, w in enumerate(waits[:-1]):
                    out.append(mybir.InstNoOp(
                        name=f"{ins.name}-waitsplit{idx_}",
                        engine=ins.engine,
                        sync_info=mybir.SyncInfo(on_wait=[w], on_update=[]),
                    ))
                si.on_wait = [waits[-1]]
                changed = True
            out.append(ins)
        if changed:
            bbw.instructions = out


def _hoist_preamble(nc):
    """Move the wait-free head of the kernel body — input-DMA issues, the
    bias memset, and the dummy-activation chain (which drags the ~1.3us ACT
    table load with it) — into the preamble block, ahead of the Tile entry
    barrier.  The entry barrier completes ~1.5us after the per-engine
    register-init moves; these instructions depend on none of the state the
    barrier protects (fresh SBUF tiles, runtime-cleared semaphores), so
    hoisting starts the input transfer and the table load that much sooner.
    Per-engine program order is preserved; cross-engine order is already
    carried by the semaphores the instructions themselves hold."""
    blocks = nc.main_func.blocks
    if len(blocks) < 2:
        return
    pre, body = blocks[0], blocks[1]

    hoist = []
    n_act = 0
    for ins in body.instructions:
        eng = ins.engine
        if isinstance(ins, mybir.InstDMACopy) and not (
                ins.sync_info and ins.sync_info.on_wait):
            hoist.append(ins)
        elif isinstance(ins, mybir.InstMemset) and eng == mybir.EngineType.DVE:
            hoist.append(ins)
        elif isinstance(ins, mybir.InstActivation) and n_act < 2:
            # the two dummy activations are the first ACT instructions
            hoist.append(ins)
            n_act += 1
    body.instructions = [i for i in body.instructions if i not in hoist]

    # Insertion point per engine: before that engine's first entry-barrier
    # event (fallback: end of preamble).
    out = []
    inserted = set()
    for ins in pre.instructions:
        if (isinstance(ins, mybir.InstEventSemaphore)
                and ins.engine not in inserted):
            out.extend(h for h in hoist if h.engine == ins.engine)
            inserted.add(ins.engine)
        out.append(ins)
    for h in hoist:
        if h.engine not in inserted:
            out.append(h)
    pre.instructions = out


def _build_program(tpc, widths, split_waits=True):
    """tpc: frame tiles per core (usually 1); widths: per-group column
    widths (each <= GROUP, even)."""
    _orig_aeb = bass.Bass.all_engine_barrier
    bass.Bass.all_engine_barrier = (
        lambda self, *, sem_only=False: _orig_aeb(self, sem_only=True))
    try:
        nc = _build_program_inner(tpc, widths)
    finally:
        bass.Bass.all_engine_barrier = _orig_aeb
    _hoist_preamble(nc)
    if split_waits:
        _split_multi_waits(nc)
    return nc


def _build_program_inner(tpc, widths):
    f32 = mybir.dt.float32
    bf16 = mybir.dt.bfloat16
    C = sum(widths)
    A_COLS = tpc * 128
    NGROUPS = tpc * len(widths)
    nW = len(widths)

    nc = bass.Bass()
    inp = nc.declare_dram_parameter("inp", [KS, A_COLS + C], bf16,
                                    isOutput=False)
    fsums = nc.declare_dram_parameter("fsums", [128, NGROUPS], f32,
                                      isOutput=True)

    # Input DMA pieces: piece 0 (SP ring) = all A tiles + the first chunk
    # (small, so its completion receipt — the ~2.4us critical-path gate for
    # the first matmul — starts as early as possible); then two-chunk pieces
    # split between the SP and GPSIMD (SWDGE) rings so transfers overlap.
    bounds = [0, A_COLS + sum(widths[0:2])]
    off = bounds[-1]
    for gi in range(2, nW):
        off += widths[gi]
        if gi + 1 < nW and widths[gi + 1] < CHUNK:
            continue  # tiny trailing group rides with its predecessor
        bounds.append(off)
    bounds[-1] = A_COLS + C
    n_pieces = len(bounds) - 1
    # Every piece pays its own ~2.5-3.5us issue->completion-sem latency, so
    # pieces issue in parallel across the SP HWDGE and GPSIMD SWDGE rings at
    # t~0: piece 0 (A + the first two groups — the pipeline head) first on
    # SP, later groups one piece each, alternating GPSIMD/SP so each ring's
    # first piece lands early.  The ACT ring stays clear: a piece there
    # would delay the sqrt table load (and its SDMA traffic delays piece 0).
    piece_engines = ["sync"] + ["gpsimd", "sync"] * n_pieces
    piece_engines = piece_engines[:n_pieces]

    # acc is split into two tiles: the first output DMA (all but the last
    # two groups) issues mid-pipeline and its ~2us HBM-write receipt hides
    # under compute; only the final ~1KB DMA's receipt sits on the tail.
    # (Tile tracks deps per-tile, so the split avoids false dependencies.)
    n_acc_a = max(1, NGROUPS - 2)
    v_cols = list(range(NGROUPS))

    with tile.TileContext(nc) as tc:
        with tc.tile_pool(name="const", bufs=1) as const_pool, \
             tc.tile_pool(name="dist", bufs=2 * NGROUPS) as dist_pool, \
             tc.tile_pool(name="ps",
                          bufs=min(NGROUPS, 8 * CHUNK // max(widths)),
                          space="PSUM") as ps:
            data = const_pool.tile([KS, A_COLS + C], bf16)
            for i in range(len(bounds) - 1):
                eng = getattr(nc, piece_engines[i])
                eng.dma_start(data[:, bounds[i]:bounds[i + 1]],
                              inp[:, bounds[i]:bounds[i + 1]])

            acc_a = const_pool.tile([128, n_acc_a], f32)
            if NGROUPS > n_acc_a:
                acc_b = const_pool.tile([128, NGROUPS - n_acc_a], f32)
            else:
                acc_b = None
            bias_t = const_pool.tile([128, 1], f32)
            scratch_t = const_pool.tile([128, 1], f32)
            # Idle DVE fills the sqrt-bias const; the dummy activation chain
            # on ACT absorbs the bias wait AND triggers the sqrt table load
            # (PWP, ~1.3us) at t~0, hidden under the input DMA.
            nc.vector.memset(bias_t[:], 0.0)
            nc.scalar.activation(scratch_t[:], bias_t[:],
                                 mybir.ActivationFunctionType.Sqrt,
                                 bias=bias_t[:, 0:1])
            nc.scalar.activation(scratch_t[:], bias_t[:],
                                 mybir.ActivationFunctionType.Sqrt,
                                 bias=bias_t[:, 0:1])

            for t in range(tpc):
                col = A_COLS
                for gi, w in enumerate(widths):
                    g = t * nW + gi
                    if gi == 0:
                        # Standalone LDWEIGHTS absorbs the DMA-queue wait on
                        # the PE so the real matmuls only wait on their PSUM
                        # slot (single-wait-per-instruction limit).
                        nc.tensor.ldweights(data[:, t * 128:t * 128 + 128])
                    d2 = ps.tile([128, w], f32, tag="d2")
                    for c0 in range(0, w, CHUNK):
                        cw = min(CHUNK, w - c0)
                        nc.tensor.matmul(
                            d2[:, c0:c0 + cw],
                            data[:, t * 128:(t + 1) * 128],
                            data[:, col + c0:col + c0 + cw],
                            start=True, stop=True,
                        )
                    # DVE clamps d2 to [0, 100] straight out of PSUM (also
                    # kills negative split residuals before the sqrt), then
                    # ACT computes sqrt with the free fused free-axis accum:
                    # min(sqrt(d2), 10) == sqrt(min(max(d2, 0), 100)).
                    clamp = dist_pool.tile([128, w], bf16, tag="clamp")
                    nc.vector.tensor_scalar(
                        out=clamp[:], in0=d2[:],
                        scalar1=0.0, scalar2=CLAMP2,
                        op0=mybir.AluOpType.max,
                        op1=mybir.AluOpType.min,
                    )
                    if g < n_acc_a:
                        acc_ap = acc_a[:, g:g + 1]
                    else:
                        acc_ap = acc_b[:, g - n_acc_a:g - n_acc_a + 1]
                    dist = dist_pool.tile([128, w], bf16, tag="dist")
                    nc.scalar.activation(
                        dist[:], clamp[:],
                        mybir.ActivationFunctionType.Sqrt,
                        bias=bias_t[:, 0:1],
                        accum_out=acc_ap,
                    )
                    col += w

            nc.sync.dma_start(fsums[:, 0:n_acc_a], acc_a[:])
            if acc_b is not None:
                nc.sync.dma_start(fsums[:, n_acc_a:], acc_b[:])
    return nc


_PROGRAM_CACHE = {}


def _get_program(tpc, widths):
    key = (tpc, tuple(widths))
    if key not in _PROGRAM_CACHE:
        _PROGRAM_CACHE[key] = _build_program(tpc, widths)
    return _PROGRAM_CACHE[key]


def kernel(pred_coords, true_coords, pred_rotation, pred_translation,
           true_rotation, true_translation, mask, **_run_kwargs):
    mask = np.asarray(mask)
    idx = np.nonzero(mask != 0)[0]
    nv = len(idx)
    if nv == 0:
        out = np.float32(0.0)
        return (out, None) if _run_kwargs else out

    B, A, Qv, P64 = _host_prep(pred_coords, true_coords, pred_rotation,
                               pred_translation, true_rotation,
                               true_translation, idx)
    C = 3 * nv

    # Frame split: device takes n_dev = tpc*1024 (one or more full 128-frame
    # tiles per core); a small overflow is summed exactly on the host.
    tpc = max(1, int(round(nv / 1024)))
    n_dev = min(nv, tpc * 1024)
    n_off = nv - n_dev

    # Column groups: two CHUNK-wide groups first (small, so the ACT/DVE
    # pipeline starts as soon as the first chunk's matmul lands), then
    # 2*CHUNK-wide groups (halved per-instruction overhead), remainder last
    # (kept even for the DVE 2-byte perf modes).
    C_pad = C + (C & 1)
    widths = []
    rest = C_pad
    while rest > 0:
        if len(widths) < 2:
            w = min(CHUNK, rest)
        else:
            w = min(2 * CHUNK, rest)
        if rest - w < 0:
            w = rest
        widths.append(w)
        rest -= w
    NGROUPS = tpc * len(widths)

    A_pad = np.zeros((tpc * 1024, KS), A.dtype)
    A_pad[:n_dev] = A[:n_dev]
    B_pad = np.zeros((KS, C_pad), B.dtype)
    B_pad[:, :C] = B

    in_maps = []
    for c in range(N_CORES):
        a_c = A_pad[c * tpc * 128:(c + 1) * tpc * 128].T  # (84, tpc*128)
        in_maps.append({"inp": np.ascontiguousarray(
            np.concatenate([a_c, B_pad], axis=1))})

    nc = _get_program(tpc, widths)
    res = run_bass_kernel_spmd(nc, in_maps, list(range(N_CORES)),
                               **_run_kwargs)

    # fsums column j = group j; every group of frame-tile t contributes to
    # the same partitions, so the sum only needs each column's tile index.
    nW = len(widths)
    col_tile = [g // nW for g in range(NGROUPS)]

    # Pad columns and pad frame rows yield d2=0 -> clamp 0 -> sqrt 0, so
    # they contribute exactly nothing; just skip the pad partitions.
    numer = 0.0
    for c in range(N_CORES):
        fs = np.asarray(res.results[c]["fsums"], np.float64)  # (128, NGROUPS)
        for j, t in enumerate(col_tile):
            f0 = (c * tpc + t) * 128
            cnt = min(128, max(0, n_dev - f0))
            if cnt == 0:
                continue
            numer += fs[:cnt, j].sum()

    if n_off:
        # Exact host sum for the overflow frames (O(n_off * 3nv) elements).
        d2 = Qv[n_dev:nv] @ P64  # (n_off, 3*nv)
        numer += np.minimum(np.sqrt(np.maximum(d2, 0.0)), CLAMP).sum()

    denom = float(nv) ** 2 * 3.0 + 1e-8
    out = np.float32(numer / denom / 10.0)
    if _run_kwargs:
        return out, res
    return out
